# revision 12
# baseline (speedup 1.0000x reference)
"""BatchHardContrastiveLoss Trainium2 kernel (8-core SPMD), v2.

Math: dist^2[i,j] = sq[i] + sq[j] - 2*x_i.x_j.  Per anchor i we need
  hardest_pos[i] = max over positives j of dist[i,j]
  hardest_neg[i] = min over negatives j of dist[i,j]
Monotonicity of sqrt lets us mine in the d2 domain.  On device we compute
  v[i,j] = s*(sq[j] - 2*g[i,j])          (PE, fp8 DoubleRow matmul + bf16
                                          sq-row matmul, fp32 PSUM)
and mine BOTH reductions in a single DVE pass per PSUM tile with a
hand-authored dual-reduction custom op (ANT_DUAL_MINMAX_REDUCE):
  pos:  running max of  v - FILL*inv_pos   (MAX scan in stage 5, emitted
        through a delay lane to a stride-0 out slot; final write wins)
  neg:  running min of  v + FILL*inv_neg   (stage-7 MIN accumulator ->
        accum_out)
where the u8 mask plane m = 2*inv_pos + inv_neg in {1,2,3} is decoded
on the fly (t1 = m>=2; pos bias = FILL*t1; neg bias = FILL*(m - 2*t1)).
Per-chunk partials land in [128, n_ib*n_jc] slots; the host merges the
n_jc chunk partials per row, decodes d2, and applies margins/AvgNonZero
(O(N) work).  FILL separates masked from unmasked values and doubles as
the empty-row detector.
"""

import numpy as np
import ml_dtypes

import concourse.bass as bass  # noqa: F401  (kept for parity with env)
import concourse.mybir as mybir
import concourse.tile as tile
from concourse import bacc
from concourse import dve_ops as _dvo
from concourse.bass_utils import run_bass_kernel_spmd
from concourse.dve_spec import C0, C1, C2, Spec, Src0, Src1, minn
from concourse.dve_table_gen import dve_ver_for
from concourse.dve_uop import (
    DISABLE,
    ENABLE,
    AluInp,
    AluOp,
    DelayInp,
    DveOpSpec,
    InpSel,
    OutPath,
    OutSel,
    Trigger,
    UopConfig,
)

N_CORES = 8
FILL = 240.0
POS_MARGIN = 0.2
NEG_MARGIN = 0.2

BF16 = mybir.dt.bfloat16
F32 = mybir.dt.float32
U8 = mybir.dt.uint8
FP8 = mybir.dt.float8e4


# --------------------------------------------------------------------------
# Dual min/max reduction custom DVE op (hand-authored uop program).
#
# Inputs:  in0 = v (PSUM fp32 [128, K]), in1 = m (SBUF u8 [128, K]),
#          s0 (CONST_0) = 2.0, s1 (CONST_1) = +3e38 (neg-min seed),
#          imm2 (CONST_2) = FILL.
# Input-mux lanes: d0=Src0(v) d1=Src1(m) d2=C0 d3=C2 d4=C1 d5=MAX_NEG.
# Steady stages (one ALU each):
#   s0: t1 = IS_GE(m, 2.0)                  (inv_pos)
#   s1: ft = t1 * FILL
#   s2: fm = m * FILL                        [capture ft -> lane1]
#   s3: w  = fm - ft                         (= FILL*(m - t1))
#   s4: pv = v - ft                          [capture w  -> lane0]
#   s5: posmax = MAX(CURR, pv)   (scan)      [capture pv -> lane2]
#   s6: nv = pv + w  (= v + FILL*(m-2*t1))   [capture posmax -> lane0]
#   s7: negmin = MIN(CURR, nv)   (accum -> out_a / accum_out)
#   WR0_LO emits DELAY_0 (running posmax); out AP is stride-0 so the
#   final element's value persists.
# Seed uop (1 bubble, no consume/write): s5 = BYPASS(lane5=MAX_NEG),
#   s7 = BYPASS(lane4=C1) initialize both recurrence flops.
# --------------------------------------------------------------------------

_ALL_LANES = (0, 1, 2, 3, 4, 5)


def _dual_steady_uop() -> UopConfig:
    u = UopConfig()
    u.enable_input(InpSel.SRC_0, 1)  # lane0 = v
    u.enable_input(InpSel.SRC_1, 2)  # lane1 = m
    u.enable_input(InpSel.CONST_0, 3)  # lane2 = 2.0
    u.enable_input(InpSel.CONST_2, 4)  # lane3 = FILL
    u.enable_input(InpSel.CONST_1, 5)  # lane4 = neg seed
    u.enable_input(InpSel.MAX_NEG, 6)  # lane5 = pos seed
    dp = u.datapath_config
    dp[0].enable_alu(
        AluOp.IS_GE, AluInp.PREV_DELAY_1, AluInp.PREV_DELAY_2
    ).pass_through_delay(*_ALL_LANES)
    dp[1].enable_alu(
        AluOp.MULTIPLY, AluInp.PREV_ALU_OUT, AluInp.PREV_DELAY_3
    ).pass_through_delay(*_ALL_LANES)
    dp[2].enable_alu(
        AluOp.MULTIPLY, AluInp.PREV_DELAY_1, AluInp.PREV_DELAY_3
    ).pass_through_delay(0, 2, 3, 4, 5).enable_delay_from_src(
        DelayInp.PREV_ALU_OUT, 1
    )
    dp[3].enable_alu(
        AluOp.SUBTRACT, AluInp.PREV_ALU_OUT, AluInp.PREV_DELAY_1
    ).pass_through_delay(*_ALL_LANES)
    dp[4].enable_alu(
        AluOp.SUBTRACT, AluInp.PREV_DELAY_0, AluInp.PREV_DELAY_1
    ).pass_through_delay(1, 2, 3, 4, 5).enable_delay_from_src(
        DelayInp.PREV_ALU_OUT, 0
    )
    dp[5].enable_alu(
        AluOp.MAX, AluInp.CURR_ALU_OUT, AluInp.PREV_ALU_OUT
    ).pass_through_delay(0, 1, 3, 4, 5).enable_delay_from_src(
        DelayInp.PREV_ALU_OUT, 2
    )
    dp[6].enable_alu(
        AluOp.ADD, AluInp.PREV_DELAY_2, AluInp.PREV_DELAY_0
    ).pass_through_delay(1, 2, 3, 4, 5).enable_delay_from_src(
        DelayInp.PREV_ALU_OUT, 0
    )
    dp[7].enable_alu(
        AluOp.MIN, AluInp.CURR_ALU_OUT, AluInp.PREV_ALU_OUT
    ).pass_through_delay(*_ALL_LANES)
    dp[7].alu_out_a_enable = ENABLE
    u.enable_output(OutSel.DELAY_0, OutPath.WR0_LO)
    u.require_inp0 = ENABLE
    u.require_inp1 = ENABLE
    u.trigger = (Trigger.SRC_TENSOR_DONE, Trigger.NONE, Trigger.NONE)
    u.next_uop = (0, 0, 0)
    u.accum_enabled = ENABLE
    return u


def _dual_seed_uop() -> UopConfig:
    u = _dual_steady_uop()
    dp = u.datapath_config
    dp[5].enable_alu(AluOp.BYPASS, AluInp.PREV_DELAY_5, AluInp.PREV_DELAY_5)
    dp[7].enable_alu(AluOp.BYPASS, AluInp.PREV_DELAY_4, AluInp.PREV_DELAY_4)
    dp[7].alu_out_a_enable = ENABLE
    for p in OutPath:
        u.out_enable[p] = DISABLE
    u.require_inp0 = DISABLE
    u.require_inp1 = DISABLE
    u.repeat_count = 1
    u.trigger = (Trigger.COUNT, Trigger.NONE, Trigger.NONE)
    u.next_uop = (1, 0, 0)
    return u


def _dual_reference(in0, in1, c0, c1, c2):
    """CoreSim reference: (out = running max of pv, accum = min of nv)."""
    v = np.asarray(in0, np.float32)
    m = np.asarray(in1, np.float32)
    t1 = (m >= c0).astype(np.float32)
    pv = v - t1 * c2
    nv = v + c2 * (m - 2.0 * t1)
    out = np.maximum.accumulate(pv, axis=-1)
    acc = np.minimum(np.min(nv, axis=-1, keepdims=True), c1)
    return out, acc


def _register_dual_op(name: str = "ANT_DUAL_MINMAX_REDUCE"):
    for op in _dvo.OPS:
        if op.name == name:
            return op
    row = _dvo._CUSTOM_DVE_ROW_BASE + len(_dvo.OPS)
    assert row < 0x20, "custom-DVE sub-opcode rows exhausted"
    _dvo._SUB_OPCODE_FOR_NAME[name] = row
    ver = dve_ver_for("TRN2")
    uops = [_dual_seed_uop(), _dual_steady_uop()]
    for u in uops:
        u.validate(ver)
    opspec = DveOpSpec(name=name, opcode=row, uops=uops, rd1_en=True)
    sha = opspec.sha(ver)
    # Carrier Spec: used only for call-site asserts (has_src1, accum present)
    # and the CoreSim reference.  The compiled uop program is pinned into the
    # compile cache below, so lower() never runs on this body.
    _t1 = Src1 >= C0
    carrier = Spec(
        body=(Src0 - _t1 * C2) + (Src1 * C2 - _t1 * C2),
        accum=minn,
        accum_init=C1,
        reference=_dual_reference,
    )
    op = _dvo.DveOp(name, carrier, subdim=False, uops_sha={ver: sha})
    _dvo._COMPILE_CACHE[(name, ver)] = opspec
    _dvo.OPS.append(op)
    _dvo.CUSTOM_DVE_SPECS[name] = carrier
    return op


DUAL_OP = _register_dual_op()


# --------------------------------------------------------------------------
# Device program
# --------------------------------------------------------------------------


def build_nc(
    R,
    N,
    D,
    jch=1024,
    seg=512,
    repeat=1,
    use_fp8=True,
    bench_skip=(),  # subset of {"pe", "dve", "mask_dma"} for ablation timing
):
    """One-core program; run SPMD on all cores with per-core inputs.

    R: anchor rows owned by this core, N: total columns, D: embed dim (256).
    repeat>1 wraps the whole body in a device-side loop (for timing by
    differencing; max/min partials are recomputed identically per repeat).
    """
    assert R % 128 == 0 and N % jch == 0 and jch % seg == 0 and D == 256
    n_ib = R // 128
    n_jc = N // jch
    n_seg = jch // seg
    n_part = n_ib * n_jc

    nc = bacc.Bacc(None, target_bir_lowering=False)
    if use_fp8:
        lhs_d = nc.dram_tensor("lhs8", [128, 2, R], FP8, kind="ExternalInput")
        rhs_d = nc.dram_tensor("rhs8", [128, 2, N], FP8, kind="ExternalInput")
    else:
        lhs_d = nc.dram_tensor("lhsT", [D, R], BF16, kind="ExternalInput")
        rhs_d = nc.dram_tensor("rhs", [D, N], BF16, kind="ExternalInput")
    sqr_d = nc.dram_tensor("sqrow", [2, N], BF16, kind="ExternalInput")
    ones_d = nc.dram_tensor("ones2", [2, 128], BF16, kind="ExternalInput")
    plane_d = nc.dram_tensor("plane", [R, N], U8, kind="ExternalInput")
    rpos_d = nc.dram_tensor("rpos", [128, n_part], F32, kind="ExternalOutput")
    rneg_d = nc.dram_tensor("rneg", [128, n_part], F32, kind="ExternalOutput")

    with tile.TileContext(nc) as tc:
        with (
            tc.tile_pool(name="const", bufs=1) as cpool,
            tc.tile_pool(name="masks", bufs=1) as mpool,
            tc.tile_pool(name="psum", bufs=4, space="PSUM") as ppool,
            tc.tile_pool(name="acc", bufs=1) as apool,
        ):
            if use_fp8:
                rhs_sb = cpool.tile([128, 2, N], FP8, tag="rhs", name="rhs_sb")
                lhs_sb = cpool.tile([128, 2, R], FP8, tag="lhs", name="lhs_sb")
                nc.sync.dma_start(rhs_sb[:], rhs_d[:])
                nc.sync.dma_start(lhs_sb[:], lhs_d[:])
                rhs_k = lhs_k = None
            else:
                rhs_k, lhs_k = [], []
                for k in range(2):
                    rk = cpool.tile([128, N], BF16, tag=f"rhs{k}", name=f"rhs_sb{k}")
                    lk = cpool.tile([128, R], BF16, tag=f"lhs{k}", name=f"lhs_sb{k}")
                    nc.sync.dma_start(rk[:], rhs_d[k * 128 : (k + 1) * 128, :])
                    nc.sync.dma_start(lk[:], lhs_d[k * 128 : (k + 1) * 128, :])
                    rhs_k.append(rk)
                    lhs_k.append(lk)
            sqr_sb = cpool.tile([2, N], BF16, tag="sqr", name="sqr_sb")
            ones_sb = cpool.tile([2, 128], BF16, tag="ones", name="ones_sb")
            nc.sync.dma_start(sqr_sb[:], sqr_d[:])
            nc.sync.dma_start(ones_sb[:], ones_d[:])

            posp = apool.tile([128, n_part], F32, tag="posp", name="posp")
            negp = apool.tile([128, n_part], F32, tag="negp", name="negp")

            # One-time setup: materialize sq128[p, j] = s*sq[j] (same for all
            # partitions) in SBUF fp32 via the ones x sqrow PE trick + ACT
            # copy.  The hot loop then ACT-preloads each PSUM tile from it so
            # PE runs only the fp8 DoubleRow matmuls (start=False accumulate).
            sq128 = cpool.tile([128, N], BF16, tag="sq128", name="sq128")
            for jc in range(n_jc):
                pts = ppool.tile([128, jch], F32, tag="pt", name=f"sqinit{jc}")
                for sg in range(n_seg):
                    j0 = jc * jch + sg * seg
                    nc.tensor.matmul(
                        pts[:, sg * seg : (sg + 1) * seg],
                        ones_sb[:],
                        sqr_sb[:, j0 : j0 + seg],
                        start=True,
                        stop=True,
                    )
                nc.scalar.copy(sq128[:, jc * jch : (jc + 1) * jch], pts[:])

            def trace_body():
                bms = []
                for ib in range(n_ib):
                    bm = mpool.tile([128, N], U8, tag=f"bm{ib}", name=f"bm{ib}")
                    if "mask_dma" not in bench_skip:
                        nc.sync.dma_start(
                            bm[:], plane_d[ib * 128 : (ib + 1) * 128, :]
                        )
                    bms.append(bm)
                for ib in range(n_ib):
                    ibsl = slice(ib * 128, (ib + 1) * 128)
                    for jc in range(n_jc):
                        pt = ppool.tile([128, jch], F32, tag="pt", name=f"pt{ib}_{jc}")
                        # ACT preloads the s*sq[j] row plane; PE accumulates
                        # the Gram term on top (start=False).  Keeps the PE
                        # burst small enough to hide under the DVE pass even
                        # at low p-state.
                        nc.scalar.copy(pt[:], sq128[:, jc * jch : (jc + 1) * jch])
                        for sg in range(n_seg):
                            if "pe" in bench_skip:
                                break
                            j0 = jc * jch + sg * seg
                            osl = slice(sg * seg, (sg + 1) * seg)
                            if use_fp8:
                                nc.tensor.matmul(
                                    pt[:, osl],
                                    lhs_sb[:, :, ibsl],
                                    rhs_sb[:, :, j0 : j0 + seg],
                                    start=False,
                                    stop=True,
                                    perf_mode=mybir.MatmulPerfMode.DoubleRow,
                                    skip_group_check=True,
                                )
                            else:
                                nc.tensor.matmul(
                                    pt[:, osl],
                                    lhs_k[0][:, ibsl],
                                    rhs_k[0][:, j0 : j0 + seg],
                                    start=False,
                                    stop=False,
                                    skip_group_check=True,
                                )
                                nc.tensor.matmul(
                                    pt[:, osl],
                                    lhs_k[1][:, ibsl],
                                    rhs_k[1][:, j0 : j0 + seg],
                                    start=False,
                                    stop=True,
                                    skip_group_check=True,
                                )
                        idx = ib * n_jc + jc
                        if "dve" in bench_skip:
                            continue
                        nc.vector._custom_dve(
                            DUAL_OP,
                            out=posp[:, idx : idx + 1].broadcast_to((128, jch)),
                            in0=pt[:],
                            in1=bms[ib][:, jc * jch : (jc + 1) * jch],
                            s0=2.0,
                            s1=3.0e38,
                            imm2=FILL,
                            accum_out=negp[:, idx : idx + 1],
                        )

            if repeat == 1:
                trace_body()
            else:
                with tc.For_i(0, repeat, 1):
                    trace_body()
            if "dve" not in bench_skip:
                nc.sync.dma_start(rpos_d[:], posp[:])
                nc.sync.dma_start(rneg_d[:], negp[:])
    nc.compile()
    return nc


# --------------------------------------------------------------------------
# Host side
# --------------------------------------------------------------------------


def _avg_nonzero(losses):
    nz = np.count_nonzero(losses > 0)
    return float(np.sum(losses) / nz) if nz > 0 else 0.0


def _pack_fp8(a2d):
    """[256, M] -> DoubleRow-packed [128, 2, M] fp8e4: out[k, i, m] = a[i*128+k, m]."""
    d, m = a2d.shape
    assert d == 256
    return np.ascontiguousarray(
        a2d.reshape(2, 128, m).transpose(1, 0, 2)
    ).astype(ml_dtypes.float8_e4m3)


def _prep_inputs(embeddings, positives_mask, negatives_mask, n_cores, use_fp8=True):
    x = np.asarray(embeddings, dtype=np.float32)
    pos = np.asarray(positives_mask).astype(bool)
    neg = np.asarray(negatives_mask).astype(bool)
    n, d = x.shape
    r = n // n_cores

    sq = np.sum(x.astype(np.float64) ** 2, axis=1)
    sq_max = float(sq.max())
    sq_min = float(sq.min())
    # scale so the full spread of v = s*(sq_j - 2g) fits inside FILL with
    # margin; |g| <= sq_max by Cauchy-Schwarz. Power of two => exact scaling.
    s = 0.125
    while s * (5.0 * sq_max - sq_min) > FILL - 32.0 and s > 2.0**-40:
        s *= 0.5

    sqs = (s * sq).astype(np.float32)
    hi = sqs.astype(ml_dtypes.bfloat16)
    lo = (sqs - hi.astype(np.float32)).astype(ml_dtypes.bfloat16)
    sqrow = np.stack([hi, lo], axis=0)  # [2, N]
    ones2 = np.ones((2, 128), dtype=ml_dtypes.bfloat16)

    # mask plane: m = 2*inv_pos + inv_neg in {1,2,3} ({0} never occurs)
    plane_full = (~pos).astype(np.uint8) * 2 + (~neg).astype(np.uint8)

    c = np.sqrt(2.0 * s)  # split -2s symmetrically across the two operands
    xT = x.T  # [D, N]
    if use_fp8:
        rhs_full = _pack_fp8(c * xT)
    else:
        rhs_full = (-2.0 * s * xT).astype(ml_dtypes.bfloat16)

    in_maps = []
    for ci in range(n_cores):
        rows = slice(ci * r, (ci + 1) * r)
        if use_fp8:
            lhs = _pack_fp8(-c * np.ascontiguousarray(xT[:, rows]))
            im = {"lhs8": lhs, "rhs8": rhs_full}
        else:
            lhs = np.ascontiguousarray(x[rows].T).astype(ml_dtypes.bfloat16)
            im = {"lhsT": lhs, "rhs": rhs_full}
        im["sqrow"] = sqrow
        im["ones2"] = ones2
        im["plane"] = plane_full[rows]
        in_maps.append(im)
    aux = {"sq": sq, "s": s, "sq_max": sq_max, "sq_min": sq_min, "n": n, "r": r}
    return in_maps, aux


def _decode(results, aux, n_cores, n_jc=8):
    sq, s = aux["sq"], aux["s"]
    n, r = aux["n"], aux["r"]
    n_ib = r // 128

    r_pos = np.empty(n, dtype=np.float64)
    r_neg = np.empty(n, dtype=np.float64)
    for c in range(n_cores):
        rp = np.asarray(results[c]["rpos"], dtype=np.float64)  # [128, n_ib*n_jc]
        rn = np.asarray(results[c]["rneg"], dtype=np.float64)
        rp = rp.reshape(128, n_ib, n_jc).max(axis=2)
        rn = rn.reshape(128, n_ib, n_jc).min(axis=2)
        base = c * r
        for ib in range(n_ib):
            r_pos[base + ib * 128 : base + (ib + 1) * 128] = rp[:, ib]
            r_neg[base + ib * 128 : base + (ib + 1) * 128] = rn[:, ib]

    d2_pos = r_pos / s + sq
    d2_neg = r_neg / s + sq
    # Masked-out entries are pushed >= FILL/s (= 32/s margin beyond any real
    # d2) away from the valid window; seeds are +-3e38.
    has_pos = d2_pos > -16.0 / s
    has_neg = d2_neg < 4.0 * aux["sq_max"] + 16.0 / s
    valid = has_pos & has_neg

    hardest_pos = np.sqrt(np.maximum(np.where(has_pos, d2_pos, 0.0), 1e-12))
    hardest_neg = np.sqrt(np.maximum(np.where(has_neg, d2_neg, 0.0), 1e-12))
    pos_loss = np.where(valid, np.maximum(hardest_pos - POS_MARGIN, 0.0), 0.0)
    neg_loss = np.where(valid, np.maximum(NEG_MARGIN - hardest_neg, 0.0), 0.0)
    return np.float32(_avg_nonzero(pos_loss) + _avg_nonzero(neg_loss))


_NC_CACHE = {}


def _kernel_impl(embeddings, positives_mask, negatives_mask, trace=False):
    x = np.asarray(embeddings)
    n, d = x.shape
    in_maps, aux = _prep_inputs(embeddings, positives_mask, negatives_mask, N_CORES)
    key = (n // N_CORES, n, d)
    if key not in _NC_CACHE:
        _NC_CACHE[key] = build_nc(*key)
    nc = _NC_CACHE[key]
    out = run_bass_kernel_spmd(nc, in_maps, list(range(N_CORES)), trace=trace)
    result = _decode(out.results, aux, N_CORES)
    return result, out


def kernel(embeddings, positives_mask, negatives_mask):
    result, _ = _kernel_impl(embeddings, positives_mask, negatives_mask)
    return result


# revision 14
# speedup vs baseline: 1.7304x; 1.7304x over previous
"""BatchHardContrastiveLoss Trainium2 kernel (8-core SPMD), v2.

Math: dist^2[i,j] = sq[i] + sq[j] - 2*x_i.x_j.  Per anchor i we need
  hardest_pos[i] = max over positives j of dist[i,j]
  hardest_neg[i] = min over negatives j of dist[i,j]
Monotonicity of sqrt lets us mine in the d2 domain.  On device we compute
  v[i,j] = s*(sq[j] - 2*g[i,j])          (PE, fp8 DoubleRow matmul + bf16
                                          sq-row matmul, fp32 PSUM)
and mine BOTH reductions in a single DVE pass per PSUM tile with a
hand-authored dual-reduction custom op (ANT_DUAL_MINMAX_REDUCE):
  pos:  running max of  v - FILL*inv_pos   (MAX scan in stage 5, emitted
        through a delay lane to a stride-0 out slot; final write wins)
  neg:  running min of  v + FILL*inv_neg   (stage-7 MIN accumulator ->
        accum_out)
where the u8 mask plane m = 2*inv_pos + inv_neg in {1,2,3} is decoded
on the fly (t1 = m>=2; pos bias = FILL*t1; neg bias = FILL*(m - 2*t1)).
Per-chunk partials land in [128, n_ib*n_jc] slots; the host merges the
n_jc chunk partials per row, decodes d2, and applies margins/AvgNonZero
(O(N) work).  FILL separates masked from unmasked values and doubles as
the empty-row detector.
"""

import numpy as np
import ml_dtypes

import concourse.bass as bass  # noqa: F401  (kept for parity with env)
import concourse.mybir as mybir
import concourse.tile as tile
from concourse import bacc
from concourse import dve_ops as _dvo
from concourse.bass_utils import run_bass_kernel_spmd
from concourse.dve_spec import C0, C1, C2, Spec, Src0, Src1, minn
from concourse.dve_table_gen import dve_ver_for
from concourse.dve_uop import (
    DISABLE,
    ENABLE,
    AluInp,
    AluOp,
    DelayInp,
    DveOpSpec,
    InpSel,
    OutPath,
    OutSel,
    Trigger,
    UopConfig,
)

N_CORES = 8
JCH = 2048
PSUM_BUFS = 2
FILL = 240.0
POS_MARGIN = 0.2
NEG_MARGIN = 0.2

BF16 = mybir.dt.bfloat16
F32 = mybir.dt.float32
U8 = mybir.dt.uint8
FP8 = mybir.dt.float8e4


# --------------------------------------------------------------------------
# Dual min/max reduction custom DVE op (hand-authored uop program).
#
# Inputs:  in0 = v (PSUM fp32 [128, K]), in1 = m (SBUF u8 [128, K]),
#          s0 (CONST_0) = 2.0, s1 (CONST_1) = +3e38 (neg-min seed),
#          imm2 (CONST_2) = FILL.
# Input-mux lanes: d0=Src0(v) d1=Src1(m) d2=C0 d3=C2 d4=C1 d5=MAX_NEG.
# Steady stages (one ALU each):
#   s0: t1 = IS_GE(m, 2.0)                  (inv_pos)
#   s1: ft = t1 * FILL
#   s2: fm = m * FILL                        [capture ft -> lane1]
#   s3: w  = fm - ft                         (= FILL*(m - t1))
#   s4: pv = v - ft                          [capture w  -> lane0]
#   s5: posmax = MAX(CURR, pv)   (scan)      [capture pv -> lane2]
#   s6: nv = pv + w  (= v + FILL*(m-2*t1))   [capture posmax -> lane0]
#   s7: negmin = MIN(CURR, nv)   (accum -> out_a / accum_out)
#   WR0_LO emits DELAY_0 (running posmax); out AP is stride-0 so the
#   final element's value persists.
# Seed uop (1 bubble, no consume/write): s5 = BYPASS(lane5=MAX_NEG),
#   s7 = BYPASS(lane4=C1) initialize both recurrence flops.
# --------------------------------------------------------------------------

_ALL_LANES = (0, 1, 2, 3, 4, 5)


def _dual_steady_uop() -> UopConfig:
    u = UopConfig()
    u.enable_input(InpSel.SRC_0, 1)  # lane0 = v
    u.enable_input(InpSel.SRC_1, 2)  # lane1 = m
    u.enable_input(InpSel.CONST_0, 3)  # lane2 = 2.0
    u.enable_input(InpSel.CONST_2, 4)  # lane3 = FILL
    u.enable_input(InpSel.CONST_1, 5)  # lane4 = neg seed
    u.enable_input(InpSel.MAX_NEG, 6)  # lane5 = pos seed
    dp = u.datapath_config
    dp[0].enable_alu(
        AluOp.IS_GE, AluInp.PREV_DELAY_1, AluInp.PREV_DELAY_2
    ).pass_through_delay(*_ALL_LANES)
    dp[1].enable_alu(
        AluOp.MULTIPLY, AluInp.PREV_ALU_OUT, AluInp.PREV_DELAY_3
    ).pass_through_delay(*_ALL_LANES)
    dp[2].enable_alu(
        AluOp.MULTIPLY, AluInp.PREV_DELAY_1, AluInp.PREV_DELAY_3
    ).pass_through_delay(0, 2, 3, 4, 5).enable_delay_from_src(
        DelayInp.PREV_ALU_OUT, 1
    )
    dp[3].enable_alu(
        AluOp.SUBTRACT, AluInp.PREV_ALU_OUT, AluInp.PREV_DELAY_1
    ).pass_through_delay(*_ALL_LANES)
    dp[4].enable_alu(
        AluOp.SUBTRACT, AluInp.PREV_DELAY_0, AluInp.PREV_DELAY_1
    ).pass_through_delay(1, 2, 3, 4, 5).enable_delay_from_src(
        DelayInp.PREV_ALU_OUT, 0
    )
    dp[5].enable_alu(
        AluOp.MAX, AluInp.CURR_ALU_OUT, AluInp.PREV_ALU_OUT
    ).pass_through_delay(0, 1, 3, 4, 5).enable_delay_from_src(
        DelayInp.PREV_ALU_OUT, 2
    )
    dp[6].enable_alu(
        AluOp.ADD, AluInp.PREV_DELAY_2, AluInp.PREV_DELAY_0
    ).pass_through_delay(1, 2, 3, 4, 5).enable_delay_from_src(
        DelayInp.PREV_ALU_OUT, 0
    )
    dp[7].enable_alu(
        AluOp.MIN, AluInp.CURR_ALU_OUT, AluInp.PREV_ALU_OUT
    ).pass_through_delay(*_ALL_LANES)
    dp[7].alu_out_a_enable = ENABLE
    u.enable_output(OutSel.DELAY_0, OutPath.WR0_LO)
    u.require_inp0 = ENABLE
    u.require_inp1 = ENABLE
    u.trigger = (Trigger.SRC_TENSOR_DONE, Trigger.NONE, Trigger.NONE)
    u.next_uop = (0, 0, 0)
    u.accum_enabled = ENABLE
    return u


def _dual_seed_uop() -> UopConfig:
    u = _dual_steady_uop()
    dp = u.datapath_config
    dp[5].enable_alu(AluOp.BYPASS, AluInp.PREV_DELAY_5, AluInp.PREV_DELAY_5)
    dp[7].enable_alu(AluOp.BYPASS, AluInp.PREV_DELAY_4, AluInp.PREV_DELAY_4)
    dp[7].alu_out_a_enable = ENABLE
    for p in OutPath:
        u.out_enable[p] = DISABLE
    u.require_inp0 = DISABLE
    u.require_inp1 = DISABLE
    u.repeat_count = 1
    u.trigger = (Trigger.COUNT, Trigger.NONE, Trigger.NONE)
    u.next_uop = (1, 0, 0)
    return u


def _dual_reference(in0, in1, c0, c1, c2):
    """CoreSim reference: (out = running max of pv, accum = min of nv)."""
    v = np.asarray(in0, np.float32)
    m = np.asarray(in1, np.float32)
    t1 = (m >= c0).astype(np.float32)
    pv = v - t1 * c2
    nv = v + c2 * (m - 2.0 * t1)
    out = np.maximum.accumulate(pv, axis=-1)
    acc = np.minimum(np.min(nv, axis=-1, keepdims=True), c1)
    return out, acc


def _register_dual_op(name: str = "ANT_DUAL_MINMAX_REDUCE"):
    for op in _dvo.OPS:
        if op.name == name:
            return op
    row = _dvo._CUSTOM_DVE_ROW_BASE + len(_dvo.OPS)
    assert row < 0x20, "custom-DVE sub-opcode rows exhausted"
    _dvo._SUB_OPCODE_FOR_NAME[name] = row
    ver = dve_ver_for("TRN2")
    uops = [_dual_seed_uop(), _dual_steady_uop()]
    for u in uops:
        u.validate(ver)
    opspec = DveOpSpec(name=name, opcode=row, uops=uops, rd1_en=True)
    sha = opspec.sha(ver)
    # Carrier Spec: used only for call-site asserts (has_src1, accum present)
    # and the CoreSim reference.  The compiled uop program is pinned into the
    # compile cache below, so lower() never runs on this body.
    _t1 = Src1 >= C0
    carrier = Spec(
        body=(Src0 - _t1 * C2) + (Src1 * C2 - _t1 * C2),
        accum=minn,
        accum_init=C1,
        reference=_dual_reference,
    )
    op = _dvo.DveOp(name, carrier, subdim=False, uops_sha={ver: sha})
    _dvo._COMPILE_CACHE[(name, ver)] = opspec
    _dvo.OPS.append(op)
    _dvo.CUSTOM_DVE_SPECS[name] = carrier
    return op


DUAL_OP = _register_dual_op()


# --------------------------------------------------------------------------
# Device program
# --------------------------------------------------------------------------


def build_nc(
    R,
    N,
    D,
    jch=JCH,
    seg=512,
    repeat=1,
    use_fp8=True,
    bench_skip=(),  # subset of {"pe", "dve", "mask_dma"} for ablation timing
):
    """One-core program; run SPMD on all cores with per-core inputs.

    R: anchor rows owned by this core, N: total columns, D: embed dim (256).
    repeat>1 wraps the whole body in a device-side loop (for timing by
    differencing; max/min partials are recomputed identically per repeat).
    """
    assert R % 128 == 0 and N % jch == 0 and jch % seg == 0 and D == 256
    n_ib = R // 128
    n_jc = N // jch
    n_seg = jch // seg
    n_part = n_ib * n_jc

    nc = bacc.Bacc(None, target_bir_lowering=False)
    if use_fp8:
        lhs_d = nc.dram_tensor("lhs8", [128, 2, R], FP8, kind="ExternalInput")
        rhs_d = nc.dram_tensor("rhs8", [128, 2, N], FP8, kind="ExternalInput")
    else:
        lhs_d = nc.dram_tensor("lhsT", [D, R], BF16, kind="ExternalInput")
        rhs_d = nc.dram_tensor("rhs", [D, N], BF16, kind="ExternalInput")
    if not use_fp8:
        sqr_d = nc.dram_tensor("sqrow", [2, N], BF16, kind="ExternalInput")
        ones_d = nc.dram_tensor("ones2", [2, 128], BF16, kind="ExternalInput")
    plane_d = nc.dram_tensor("plane", [R, N], U8, kind="ExternalInput")
    rpos_d = nc.dram_tensor("rpos", [128, n_part], F32, kind="ExternalOutput")
    rneg_d = nc.dram_tensor("rneg", [128, n_part], F32, kind="ExternalOutput")

    with tile.TileContext(nc) as tc:
        with (
            tc.tile_pool(name="const", bufs=1) as cpool,
            tc.tile_pool(name="masks", bufs=1) as mpool,
            tc.tile_pool(name="psum", bufs=PSUM_BUFS, space="PSUM") as ppool,
            tc.tile_pool(name="acc", bufs=1) as apool,
        ):
            if use_fp8:
                rhs_sb = cpool.tile([128, 2, N], FP8, tag="rhs", name="rhs_sb")
                lhs_sb = cpool.tile([128, 2, R], FP8, tag="lhs", name="lhs_sb")
                nc.sync.dma_start(rhs_sb[:], rhs_d[:])
                nc.sync.dma_start(lhs_sb[:], lhs_d[:])
                rhs_k = lhs_k = None
            else:
                rhs_k, lhs_k = [], []
                for k in range(2):
                    rk = cpool.tile([128, N], BF16, tag=f"rhs{k}", name=f"rhs_sb{k}")
                    lk = cpool.tile([128, R], BF16, tag=f"lhs{k}", name=f"lhs_sb{k}")
                    nc.sync.dma_start(rk[:], rhs_d[k * 128 : (k + 1) * 128, :])
                    nc.sync.dma_start(lk[:], lhs_d[k * 128 : (k + 1) * 128, :])
                    rhs_k.append(rk)
                    lhs_k.append(lk)
            if not use_fp8:
                sqr_sb = cpool.tile([2, N], BF16, tag="sqr", name="sqr_sb")
                ones_sb = cpool.tile([2, 128], BF16, tag="ones", name="ones_sb")
                nc.sync.dma_start(sqr_sb[:], sqr_d[:])
                nc.sync.dma_start(ones_sb[:], ones_d[:])

            posp = apool.tile([128, n_part], F32, tag="posp", name="posp")
            negp = apool.tile([128, n_part], F32, tag="negp", name="negp")

            if not use_fp8:
                # bf16 fallback: materialize sq128[p, j] = s*sq[j] via the
                # ones x sqrow PE trick + ACT copy; the hot loop ACT-preloads
                # each PSUM tile from it (matmuls accumulate with
                # start=False).  The fp8 path instead carries s*sq[j] inside
                # the DoubleRow matmul as hi/lo fp8 rows (PCA-rotated
                # embeddings free the two contraction slots).
                sq128 = cpool.tile([128, N], BF16, tag="sq128", name="sq128")
                for jc in range(n_jc):
                    pts = ppool.tile([128, jch], F32, tag="pt", name=f"sqinit{jc}")
                    for sg in range(n_seg):
                        j0 = jc * jch + sg * seg
                        nc.tensor.matmul(
                            pts[:, sg * seg : (sg + 1) * seg],
                            ones_sb[:],
                            sqr_sb[:, j0 : j0 + seg],
                            start=True,
                            stop=True,
                        )
                    nc.scalar.copy(sq128[:, jc * jch : (jc + 1) * jch], pts[:])

            def trace_body():
                bms = []
                for ib in range(n_ib):
                    bm = mpool.tile([128, N], U8, tag=f"bm{ib}", name=f"bm{ib}")
                    if "mask_dma" not in bench_skip:
                        nc.sync.dma_start(
                            bm[:], plane_d[ib * 128 : (ib + 1) * 128, :]
                        )
                    bms.append(bm)
                for ib in range(n_ib):
                    ibsl = slice(ib * 128, (ib + 1) * 128)
                    for jc in range(n_jc):
                        pt = ppool.tile([128, jch], F32, tag="pt", name=f"pt{ib}_{jc}")
                        if not use_fp8:
                            nc.scalar.copy(
                                pt[:], sq128[:, jc * jch : (jc + 1) * jch]
                            )
                        if "pe" in bench_skip and use_fp8:
                            # keep pt written for tile dep tracking
                            nc.scalar.memzero(pt[:])
                        for sg in range(n_seg):
                            if "pe" in bench_skip:
                                break
                            j0 = jc * jch + sg * seg
                            osl = slice(sg * seg, (sg + 1) * seg)
                            if use_fp8:
                                nc.tensor.matmul(
                                    pt[:, osl],
                                    lhs_sb[:, :, ibsl],
                                    rhs_sb[:, :, j0 : j0 + seg],
                                    start=True,
                                    stop=True,
                                    perf_mode=mybir.MatmulPerfMode.DoubleRow,
                                )
                            else:
                                nc.tensor.matmul(
                                    pt[:, osl],
                                    lhs_k[0][:, ibsl],
                                    rhs_k[0][:, j0 : j0 + seg],
                                    start=False,
                                    stop=False,
                                    skip_group_check=True,
                                )
                                nc.tensor.matmul(
                                    pt[:, osl],
                                    lhs_k[1][:, ibsl],
                                    rhs_k[1][:, j0 : j0 + seg],
                                    start=False,
                                    stop=True,
                                    skip_group_check=True,
                                )
                        idx = ib * n_jc + jc
                        if "dve" in bench_skip:
                            continue
                        nc.vector._custom_dve(
                            DUAL_OP,
                            out=posp[:, idx : idx + 1].broadcast_to((128, jch)),
                            in0=pt[:],
                            in1=bms[ib][:, jc * jch : (jc + 1) * jch],
                            s0=2.0,
                            s1=3.0e38,
                            imm2=FILL,
                            accum_out=negp[:, idx : idx + 1],
                        )

            if repeat == 1:
                trace_body()
            else:
                with tc.For_i(0, repeat, 1):
                    trace_body()
            if "dve" not in bench_skip:
                nc.sync.dma_start(rpos_d[:], posp[:])
                nc.sync.dma_start(rneg_d[:], negp[:])
    nc.compile()
    return nc


# --------------------------------------------------------------------------
# Host side
# --------------------------------------------------------------------------


def _avg_nonzero(losses):
    nz = np.count_nonzero(losses > 0)
    return float(np.sum(losses) / nz) if nz > 0 else 0.0


def _pack_fp8(a2d):
    """[256, M] -> DoubleRow-packed [128, 2, M] fp8e4: out[k, i, m] = a[i*128+k, m]."""
    d, m = a2d.shape
    assert d == 256
    return np.ascontiguousarray(
        a2d.reshape(2, 128, m).transpose(1, 0, 2)
    ).astype(ml_dtypes.float8_e4m3)


def _prep_inputs(embeddings, positives_mask, negatives_mask, n_cores, use_fp8=True):
    x = np.asarray(embeddings, dtype=np.float32)
    pos = np.asarray(positives_mask).astype(bool)
    neg = np.asarray(negatives_mask).astype(bool)
    n, d = x.shape
    r = n // n_cores

    sq = np.sum(x.astype(np.float64) ** 2, axis=1)
    sq_max = float(sq.max())
    sq_min = float(sq.min())
    # scale so the full spread of v = s*(sq_j - 2g) fits inside FILL with
    # margin; |g| <= sq_max by Cauchy-Schwarz. Power of two => exact scaling.
    s = 0.125
    while s * (5.0 * sq_max - sq_min) > FILL - 32.0 and s > 2.0**-40:
        s *= 0.5

    # mask plane: m = 2*inv_pos + inv_neg in {1,2,3} ({0} never occurs)
    plane_full = (~pos).astype(np.uint8) * 2 + (~neg).astype(np.uint8)

    c = np.sqrt(2.0 * s)  # split -2s symmetrically across the two operands
    if use_fp8:
        # Rotate x into its PCA basis (rotation preserves all pairwise
        # distances exactly), drop the two lowest-variance directions, and
        # use the freed contraction slots to carry s*sq[j] as an fp8 hi/lo
        # pair (stationary side holds ones).  PSUM then directly receives
        # v = s*(sq_j - 2*g) from a single DoubleRow matmul per segment.
        x64 = x.astype(np.float64)
        _, V = np.linalg.eigh(x64.T @ x64)  # ascending eigenvalues
        xr = x64 @ V[:, 2:]  # [N, D-2], dims sorted low->high variance
        f8 = ml_dtypes.float8_e4m3
        sqs = (s * sq).astype(np.float32)
        sq_hi = sqs.astype(f8)
        sq_lo = (sqs - sq_hi.astype(np.float32)).astype(f8)
        rhs_aug = np.empty((d, n), dtype=np.float32)
        rhs_aug[: d - 2] = (c * xr.T).astype(f8).astype(np.float32)
        rhs_aug[d - 2] = sq_hi.astype(np.float32)
        rhs_aug[d - 1] = sq_lo.astype(np.float32)
        rhs_full = _pack_fp8(rhs_aug)
        lhs_aug_full = np.empty((d, n), dtype=np.float32)
        lhs_aug_full[: d - 2] = (-c * xr.T).astype(f8).astype(np.float32)
        lhs_aug_full[d - 2 :] = 1.0
    else:
        sqs = (s * sq).astype(np.float32)
        hi = sqs.astype(ml_dtypes.bfloat16)
        lo = (sqs - hi.astype(np.float32)).astype(ml_dtypes.bfloat16)
        sqrow = np.stack([hi, lo], axis=0)  # [2, N]
        ones2 = np.ones((2, 128), dtype=ml_dtypes.bfloat16)
        rhs_full = (-2.0 * s * x.T).astype(ml_dtypes.bfloat16)

    in_maps = []
    for ci in range(n_cores):
        rows = slice(ci * r, (ci + 1) * r)
        if use_fp8:
            lhs = _pack_fp8(np.ascontiguousarray(lhs_aug_full[:, rows]))
            im = {"lhs8": lhs, "rhs8": rhs_full}
        else:
            lhs = np.ascontiguousarray(x[rows].T).astype(ml_dtypes.bfloat16)
            im = {"lhsT": lhs, "rhs": rhs_full, "sqrow": sqrow, "ones2": ones2}
        im["plane"] = plane_full[rows]
        in_maps.append(im)
    aux = {"sq": sq, "s": s, "sq_max": sq_max, "sq_min": sq_min, "n": n, "r": r}
    return in_maps, aux


def _decode(results, aux, n_cores, n_jc=None):
    sq, s = aux["sq"], aux["s"]
    n, r = aux["n"], aux["r"]
    n_ib = r // 128
    if n_jc is None:
        n_jc = n // JCH

    r_pos = np.empty(n, dtype=np.float64)
    r_neg = np.empty(n, dtype=np.float64)
    for c in range(n_cores):
        rp = np.asarray(results[c]["rpos"], dtype=np.float64)  # [128, n_ib*n_jc]
        rn = np.asarray(results[c]["rneg"], dtype=np.float64)
        rp = rp.reshape(128, n_ib, n_jc).max(axis=2)
        rn = rn.reshape(128, n_ib, n_jc).min(axis=2)
        base = c * r
        for ib in range(n_ib):
            r_pos[base + ib * 128 : base + (ib + 1) * 128] = rp[:, ib]
            r_neg[base + ib * 128 : base + (ib + 1) * 128] = rn[:, ib]

    d2_pos = r_pos / s + sq
    d2_neg = r_neg / s + sq
    # Masked-out entries are pushed >= FILL/s (= 32/s margin beyond any real
    # d2) away from the valid window; seeds are +-3e38.
    has_pos = d2_pos > -16.0 / s
    has_neg = d2_neg < 4.0 * aux["sq_max"] + 16.0 / s
    valid = has_pos & has_neg

    hardest_pos = np.sqrt(np.maximum(np.where(has_pos, d2_pos, 0.0), 1e-12))
    hardest_neg = np.sqrt(np.maximum(np.where(has_neg, d2_neg, 0.0), 1e-12))
    pos_loss = np.where(valid, np.maximum(hardest_pos - POS_MARGIN, 0.0), 0.0)
    neg_loss = np.where(valid, np.maximum(NEG_MARGIN - hardest_neg, 0.0), 0.0)
    return np.float32(_avg_nonzero(pos_loss) + _avg_nonzero(neg_loss))


_NC_CACHE = {}


def _kernel_impl(embeddings, positives_mask, negatives_mask, trace=False):
    x = np.asarray(embeddings)
    n, d = x.shape
    in_maps, aux = _prep_inputs(embeddings, positives_mask, negatives_mask, N_CORES)
    key = (n // N_CORES, n, d)
    if key not in _NC_CACHE:
        _NC_CACHE[key] = build_nc(*key)
    nc = _NC_CACHE[key]
    out = run_bass_kernel_spmd(nc, in_maps, list(range(N_CORES)), trace=trace)
    result = _decode(out.results, aux, N_CORES)
    return result, out


def kernel(embeddings, positives_mask, negatives_mask):
    result, _ = _kernel_impl(embeddings, positives_mask, negatives_mask)
    return result
